# revision 26
# baseline (speedup 1.0000x reference)
"""Trainium2 Bass kernel for the ContextAwareModel (bidirectional-weights LSTM).

Model (see reference): tokens [B,T] -> emb lookup -> two LSTM cells (fwd/bwd
weights, BOTH run forward in time) -> concat hidden -> pick h at target_idx
-> linear -> sigmoid.

Strategy (v2, transposed gates + 2 pipelined chains per core):
  - 8 cores = 4 batch shards (64 rows each) x 2 directions.
  - Phase A: P = emb @ w_ih_dir.T + bias -> bf16 DRAM table [VPAD, 1024],
    gate-chunk column order [f f g g o o i i] with the g block pre-scaled
    by 2 (tanh(g) = 2*sigmoid(2g) - 1). Bias is folded in via a K=1
    ones-matmul; PSUM->SBUF evacuation runs on GpSimd to keep DVE free.
  - Phase B: the 64 rows split into two independent 32-row chains that are
    software-pipelined; everything lives in a TRANSPOSED layout
    (partition = hidden/gate unit mod 128, columns = (chunk, batch)), so
    the recurrence needs zero PE transposes:
      gates^T [128, 256] PSUM = xp^T (8 seed matmuls with lhsT=xp) +
        W_hh^T h (16 matmuls with lhsT=whh chunk, rhs=h^T slice)
      s = sigmoid(gates)  (one [128,256] Act op; g block pre-scaled)
      fc = f*c; w = (sg-0.5)*i; c' = 2w+fc  (scalar_tensor_tensor fusions)
      tc = tanh(c'); h^T = o*tc -> staged bf16, streamed to DRAM
  - Tokens for both steps of a "superstep" are gathered in ONE indirect
    DMA (128 indices) to halve the SWDGE fixed cost.
  - Host gathers rows at target_idx and runs the tiny output projection.
"""

import sys

for _p in ("/opt/trn_rl_repo",):
    if _p not in sys.path:
        sys.path.insert(0, _p)

import numpy as np
import ml_dtypes

import concourse.bass as bass
import concourse.bacc as bacc
import concourse.mybir as mybir
import concourse.tile as tile
from concourse.masks import make_identity

F32 = mybir.dt.float32
BF16 = mybir.dt.bfloat16
I32 = mybir.dt.int32
AF = mybir.ActivationFunctionType
ALU = mybir.AluOpType

H = 256
E = 768
V = 7987
B = 256
T = 512
G4 = 4 * H  # 1024
BS = 64  # batch rows per core
CH = 32  # rows per chain (2 chains per core)
VC = 63  # ceil(V/128)
VPAD = VC * 128  # 8064
NCORES = 8

# Gate-chunk order in the transposed layout: [f f | g g | o o | i i]
# (chunks of 128 gate units).  PyTorch row order is [i, f, g, o].
_PERM = np.concatenate(
    [np.arange(256, 512), np.arange(512, 768), np.arange(768, 1024), np.arange(0, 256)]
)
_GSCL = np.ones(G4, dtype=np.float32)
_GSCL[256:512] = 2.0  # the g block (post-perm cols 256:512) pre-scaled by 2


def build_program(t_steps: int = T, chunk_sched=None, debug: bool = False):
    """chunk_sched[sp] = cumulative #vocab-chunks that must be emitted before
    superstep sp.  None = build the whole table up front."""
    nsup = t_steps // 2
    if chunk_sched is None:
        chunk_sched = [VC] * nsup

    nc = bacc.Bacc("TRN2", target_bir_lowering=False)
    if debug:
        dbg_xp = nc.declare_dram_parameter("dbg_xp", [128, G4], BF16, isOutput=True)
        dbg_s = nc.declare_dram_parameter("dbg_s", [4, 128, 256], F32, isOutput=True)
        dbg_c = nc.declare_dram_parameter("dbg_c", [4, 128, 64], F32, isOutput=True)

    tokg_p = nc.declare_dram_parameter("tokg", [128, nsup], I32, isOutput=False)
    embt_p = nc.declare_dram_parameter("embt", [VC, 128, 6 * 128], BF16, isOutput=False)
    wih_p = nc.declare_dram_parameter("wiht", [128, 6 * G4], BF16, isOutput=False)
    bias_p = nc.declare_dram_parameter("biasrow", [1, G4], BF16, isOutput=False)
    whh_p = nc.declare_dram_parameter("whht", [128, 2 * G4], BF16, isOutput=False)
    hout_p = nc.declare_dram_parameter("hout", [nsup, 128, 256], BF16, isOutput=True)
    p_tab = nc.dram_tensor("ptab", [VPAD, G4], BF16)

    with tile.TileContext(nc) as tc:
        with (
            tc.tile_pool(name="const", bufs=1) as cpool,
            tc.tile_pool(name="emba", bufs=3) as embp,
            tc.tile_pool(name="pouta", bufs=3) as poutp,
            tc.tile_pool(name="psa", bufs=2, space="PSUM") as psa,
            tc.tile_pool(name="xp", bufs=4) as xpp,
            tc.tile_pool(name="gps", bufs=2, space="PSUM") as gpsp,
            tc.tile_pool(name="sp", bufs=4) as spp,
            tc.tile_pool(name="dv", bufs=4) as dvp,
            tc.tile_pool(name="cs", bufs=4) as csp,
            tc.tile_pool(name="stg", bufs=3) as stgp,
        ):
            # ---- resident constants ----
            wih_sb = cpool.tile([128, 6 * G4], BF16)
            nc.sync.dma_start(wih_sb[:], wih_p[:])
            bias_sb = cpool.tile([1, G4], BF16)
            nc.sync.dma_start(bias_sb[:], bias_p[:])
            whh_sb = cpool.tile([128, 2 * G4], BF16)
            nc.sync.dma_start(whh_sb[:], whh_p[:])
            tokg_sb = cpool.tile([128, nsup], I32)
            nc.sync.dma_start(tokg_sb[:], tokg_p[:])
            # 64x64 identity replicated at partition bases 0 and 64: seed
            # matmuls run with K=64 (lhsT = a 64-row half of xp2, legal base
            # partitions 0/64) and the rhs column block selects which 32 rows
            # (chain) get extracted.
            identf = cpool.tile([128, 64], F32)
            for q in range(2):
                make_identity(nc, identf[q * 64 : (q + 1) * 64, :])
            identb = cpool.tile([128, 64], BF16)
            nc.vector.tensor_copy(identb[:], identf[:])
            ones1 = cpool.tile([1, 128], BF16)
            nc.gpsimd.memset(ones1[:], 1.0)
            zlhs = cpool.tile([1, 128], BF16)
            nc.gpsimd.memset(zlhs[:], 0.0)
            zrhs = cpool.tile([1, 256], BF16)
            nc.gpsimd.memset(zrhs[:], 0.0)

            # initial state: h = 0 (one staging-like tile), c = 0 per chain
            hzero = cpool.tile([128, 256], BF16)
            nc.gpsimd.memset(hzero[:], 0.0)
            czero = [cpool.tile([128, 64], F32, name=f"czero{i}") for i in range(2)]
            for z in czero:
                nc.gpsimd.memset(z[:], 0.0)

            # ---- table chunk emission (phase A work unit) ----
            def emit_chunk(vc):
                lhs = embp.tile([128, 6 * 128], BF16, tag="emb")
                nc.sync.dma_start(lhs[:], embt_p[vc])
                for ns in range(2):
                    ps = psa.tile([128, 512], F32, tag="pps")
                    # bias via K=1 ones-matmul (also sets has_written)
                    nc.tensor.matmul(
                        ps[:],
                        ones1[:],
                        bias_sb[:, ns * 512 : (ns + 1) * 512],
                        start=True,
                        stop=False,
                    )
                    for kc in range(6):
                        nc.tensor.matmul(
                            ps[:],
                            lhs[:, kc * 128 : (kc + 1) * 128],
                            wih_sb[:, kc * G4 + ns * 512 : kc * G4 + ns * 512 + 512],
                            start=False,
                            stop=(kc == 5),
                        )
                    po = poutp.tile([128, 512], BF16, tag="pout")
                    nc.vector.tensor_copy(po[:], ps[:])
                    nc.sync.dma_start(
                        p_tab[vc * 128 : (vc + 1) * 128, ns * 512 : (ns + 1) * 512],
                        po[:],
                    )

            # ---- phase B: software-pipelined, chain B offset half a step ----
            # per-chain state: (tile, col) for h^T, tile for c
            ht_prev = [(hzero, 0), (hzero, 64)]
            c_prev = [czero[0], czero[1]]
            chunks_done = 0
            PREFETCH = 2
            xp_tiles = {}
            stag_tiles = {}
            s_tiles = {}

            def emit_gather(spi):
                if spi >= nsup:
                    return
                xp2 = xpp.tile([128, G4], BF16, tag="xp")
                nc.gpsimd.indirect_dma_start(
                    out=xp2[:],
                    out_offset=None,
                    in_=p_tab[:, :],
                    in_offset=bass.IndirectOffsetOnAxis(
                        ap=tokg_sb[:, spi : spi + 1], axis=0
                    ),
                )
                xp_tiles[spi] = xp2

            def head(ch, t):
                """PSUM gates + sigmoid for chain ch, step t."""
                sp, ti = t // 2, t % 2
                xp2 = xp_tiles[sp]
                hbase = ti * 64
                g = gpsp.tile([128, 256], F32, tag=f"g{ch}", name=f"g{ch}_{t}")
                # exactly ONE start=True per PSUM bank (marks the whole 2KB
                # zero-region pending); later matmuls first-touch-replace,
                # then accumulate
                for c in range(8):
                    nc.tensor.matmul(
                        g[:, c * 32 : (c + 1) * 32],
                        xp2[hbase : hbase + 64, c * 128 : (c + 1) * 128],
                        identb[hbase : hbase + 64, ch * 32 : ch * 32 + 32],
                        start=(c == 0),
                        stop=False,
                        skip_group_check=True,
                    )
                htile, hcol = ht_prev[ch]
                for c in range(8):
                    for kc in range(2):
                        nc.tensor.matmul(
                            g[:, c * 32 : (c + 1) * 32],
                            whh_sb[:, kc * G4 + c * 128 : kc * G4 + c * 128 + 128],
                            htile[:, hcol + kc * 32 : hcol + kc * 32 + 32],
                            start=False,
                            stop=(c == 7 and kc == 1),
                            skip_group_check=True,
                        )
                s = spp.tile([128, 256], F32, tag=f"s{ch}", name=f"s{ch}_{t}")
                nc.scalar.activation(s[:], g[:], AF.Sigmoid)
                s_tiles[(ch, t)] = s
                if debug and sp == 0:
                    nc.sync.dma_start(dbg_s[ti * 2 + ch], s[:])

            def tail(ch, t):
                """c/h update for chain ch, step t; h lands in the stage tile."""
                sp, ti = t // 2, t % 2
                s = s_tiles.pop((ch, t))
                fc = dvp.tile([128, 64], F32, tag=f"fc{ch}", name=f"fc{ch}_{t}")
                nc.vector.tensor_tensor(fc[:], s[:, 0:64], c_prev[ch][:], ALU.mult)
                w = dvp.tile([128, 64], F32, tag=f"w{ch}", name=f"w{ch}_{t}")
                nc.vector.scalar_tensor_tensor(
                    w[:], s[:, 64:128], 0.5, s[:, 192:256], ALU.subtract, ALU.mult
                )
                cn = csp.tile([128, 64], F32, tag=f"c{ch}", name=f"c{ch}_{t}")
                nc.vector.scalar_tensor_tensor(
                    cn[:], w[:], 2.0, fc[:], ALU.mult, ALU.add
                )
                if debug and sp == 0:
                    nc.sync.dma_start(dbg_c[ti * 2 + ch], cn[:])
                tcn = dvp.tile([128, 64], F32, tag=f"tc{ch}", name=f"tc{ch}_{t}")
                nc.scalar.activation(tcn[:], cn[:], AF.Tanh)
                if sp not in stag_tiles:
                    stag_tiles[sp] = stgp.tile(
                        [128, 256], BF16, tag="stg", name=f"stg_{sp}"
                    )
                stag = stag_tiles[sp]
                slot = ti * 2 + ch
                nc.vector.tensor_tensor(
                    stag[:, slot * 64 : slot * 64 + 64],
                    s[:, 128:192],
                    tcn[:],
                    ALU.mult,
                )
                ht_prev[ch] = (stag, slot * 64)
                c_prev[ch] = cn

            A, Bc = 0, 1
            next_gather = 0

            def advance_gathers(upto):
                nonlocal chunks_done, next_gather
                while next_gather <= min(upto, nsup - 1):
                    need = chunk_sched[next_gather]
                    while chunks_done < need:
                        emit_chunk(chunks_done)
                        chunks_done += 1
                    emit_gather(next_gather)
                    next_gather += 1

            for t in range(t_steps + 1):
                if t < t_steps and t % 2 == 0:
                    advance_gathers(t // 2 + PREFETCH)
                if t == 0:
                    # table rows for the prefetched gathers are emitted
                    # first; then the pipeline prologue
                    head(A, 0)
                if t >= 1:
                    tail(Bc, t - 1)
                    if (t - 1) % 2 == 1:
                        spd = (t - 1) // 2
                        nc.sync.dma_start(hout_p[spd], stag_tiles.pop(spd)[:])
                        xp_tiles.pop(spd, None)
                if t < t_steps:
                    tail(A, t)
                    head(Bc, t)
                    if t + 1 < t_steps:
                        head(A, t + 1)

    nc.compile()
    return nc


def _plan_schedule(inputs, t_steps: int = T, ahead: int = 4):
    """First-use vocab ranking per shard + a shared chunk schedule.

    Returns (chunk_sched, ranks) where ranks[shard][v] = table row of vocab v
    for that shard's cores, and chunk_sched[sp] = cumulative chunks that must
    be emitted before superstep sp (covering every core's tokens through
    steps 2*(sp+ahead)+1).
    """
    nsup = t_steps // 2
    tokens = np.asarray(inputs["input_tensor"]).astype(np.int64)  # [B, T]
    nshard = B // BS
    ranks = []
    needs = np.zeros((nshard, t_steps), dtype=np.int64)  # distinct tokens thru t
    for s in range(nshard):
        tok = tokens[s * BS : (s + 1) * BS, :t_steps]  # [64, t]
        flat = tok.T.reshape(-1)  # step-major
        first_pos = np.full(VPAD, np.iinfo(np.int64).max, dtype=np.int64)
        # first occurrence position of each vocab id
        uniq, idx = np.unique(flat, return_index=True)
        first_pos[uniq] = idx
        order = np.argsort(first_pos, kind="stable")  # used vocab first
        rank = np.empty(VPAD, dtype=np.int64)
        rank[order] = np.arange(VPAD)
        ranks.append(rank)
        # need(t) = #vocab with first_pos < 64*(t+1)
        fp_used = np.sort(first_pos[uniq])
        needs[s] = np.searchsorted(fp_used, BS * (np.arange(t_steps) + 1))
    need_max = needs.max(axis=0)  # [t]
    chunk_sched = []
    for sp in range(nsup):
        t_cov = min(t_steps - 1, 2 * (sp + ahead) + 1)
        chunk_sched.append(min(VC, int(-(-need_max[t_cov] // 128)) + 1))
    return chunk_sched, ranks


def _prep_core_inputs(inputs, t_steps: int = T, ranks=None):
    """Build per-core input maps. Core c: dir = c % 2, shard = c // 2."""
    nsup = t_steps // 2
    tokens = np.asarray(inputs["input_tensor"]).astype(np.int32)  # [B, T]
    emb = np.asarray(inputs["emb"], dtype=np.float32)  # [V, E]

    embp = np.zeros((VPAD, E), dtype=np.float32)
    embp[:V] = emb

    def make_embt(ep):
        # embt[vc, p, kc*128+m] = ep[vc*128+m, kc*128+p]
        return np.ascontiguousarray(
            ep.reshape(VC, 128, 6, 128).transpose(0, 3, 2, 1).reshape(VC, 128, 6 * 128)
        ).astype(ml_dtypes.bfloat16)

    if ranks is None:
        embt_by_shard = [make_embt(embp)] * (B // BS)
    else:
        embt_by_shard = []
        for s in range(B // BS):
            order = np.argsort(ranks[s], kind="stable")  # order[r] = vocab at row r
            embt_by_shard.append(make_embt(embp[order]))

    per_dir = {}
    for d, sfx in ((0, "f"), (1, "b")):
        w_ih = np.asarray(inputs[f"w_ih_{sfx}"], dtype=np.float32)[_PERM]
        w_hh = np.asarray(inputs[f"w_hh_{sfx}"], dtype=np.float32)[_PERM]
        bias = (
            np.asarray(inputs[f"b_ih_{sfx}"], dtype=np.float32)
            + np.asarray(inputs[f"b_hh_{sfx}"], dtype=np.float32)
        )[_PERM]
        w_ih = w_ih * _GSCL[:, None]
        w_hh = w_hh * _GSCL[:, None]
        bias = bias * _GSCL
        # wiht[p, kc*G4+j] = w_ih[j, kc*128+p]
        wiht = np.ascontiguousarray(
            w_ih.T.reshape(6, 128, G4).transpose(1, 0, 2).reshape(128, 6 * G4)
        ).astype(ml_dtypes.bfloat16)
        # whht[k, kc*G4+j] = w_hh[j, kc*128+k]
        whht = np.ascontiguousarray(
            w_hh.T.reshape(2, 128, G4).transpose(1, 0, 2).reshape(128, 2 * G4)
        ).astype(ml_dtypes.bfloat16)
        biasrow = np.ascontiguousarray(bias[None, :]).astype(ml_dtypes.bfloat16)
        per_dir[d] = (wiht, whht, biasrow)

    in_maps = []
    for c in range(NCORES):
        d, s = c % 2, c // 2
        wiht, whht, biasrow = per_dir[d]
        tok = tokens[s * BS : (s + 1) * BS, :t_steps].astype(np.int64)  # [64, t]
        if ranks is not None:
            tok = ranks[s][tok]
        # tokg[p, sp]: idx p of superstep sp = [rows tok(2sp); rows tok(2sp+1)]
        tokg = np.ascontiguousarray(tok.T.reshape(nsup, 128).T).astype(np.int32)
        in_maps.append(
            {
                "tokg": tokg,
                "embt": embt_by_shard[s],
                "wiht": wiht,
                "biasrow": biasrow,
                "whht": whht,
            }
        )
    return in_maps


_PROGRAM_CACHE = {}
INTERLEAVE = True  # build the vocab table concurrently with the recurrence


def run_on_hw(inputs, **spmd_kwargs):
    """Run the SPMD program; returns BassKernelResults."""
    from concourse.bass_utils import run_bass_kernel_spmd

    t_steps = T
    assert int(np.asarray(inputs["max_length"])) == T

    if INTERLEAVE:
        chunk_sched, ranks = _plan_schedule(inputs, t_steps)
    else:
        chunk_sched, ranks = None, None

    key = (t_steps, tuple(chunk_sched) if chunk_sched else None)
    if key not in _PROGRAM_CACHE:
        _PROGRAM_CACHE[key] = build_program(t_steps, chunk_sched)
    nc = _PROGRAM_CACHE[key]

    in_maps = _prep_core_inputs(inputs, t_steps, ranks)
    res = run_bass_kernel_spmd(nc, in_maps, list(range(NCORES)), **spmd_kwargs)
    return res


def postprocess(inputs, res) -> np.ndarray:
    tgt = np.asarray(inputs["target_idx"]).astype(np.int64)  # [B]
    w_out = np.asarray(inputs["w_out"], dtype=np.float32)  # [1, 2H]
    b_out = np.asarray(inputs["b_out"], dtype=np.float32)  # [1]

    gathered = np.empty((B, 2 * H), dtype=np.float32)
    for c in range(NCORES):
        d, s = c % 2, c // 2
        # hout[sp, p, (ti*2+ch)*64 + kc*32 + b] = h[ch*32+b, kc*128+p] at t=2sp+ti
        hout = np.asarray(res.results[c]["hout"]).astype(np.float32)  # [nsup,128,256]
        hr = hout.reshape(T // 2, 128, 2, 2, 2, 32)  # [sp, p, ti, ch, kc, b]
        for r in range(BS):
            t = tgt[s * BS + r]
            spi, ti = int(t) // 2, int(t) % 2
            ch, b = r // 32, r % 32
            # h vector: hidden = kc*128+p
            hv = hr[spi, :, ti, ch, :, b]  # [p, kc]
            gathered[s * BS + r, d * H : (d + 1) * H] = hv.T.reshape(H)

    logits = gathered @ w_out.T + b_out  # [B, 1]
    out = 1.0 / (1.0 + np.exp(-logits))
    return out[:, None, :].astype(np.float32)  # [B, 1, 1]


def kernel(**inputs) -> np.ndarray:
    res = run_on_hw(inputs)
    return postprocess(inputs, res)


# revision 30
# speedup vs baseline: 1.0343x; 1.0343x over previous
"""Trainium2 Bass kernel for the ContextAwareModel (bidirectional-weights LSTM).

Model (see reference): tokens [B,T] -> emb lookup -> two LSTM cells (fwd/bwd
weights, BOTH run forward in time) -> concat hidden -> pick h at target_idx
-> linear -> sigmoid.

Strategy (v2, transposed gates + 2 pipelined chains per core):
  - 8 cores = 4 batch shards (64 rows each) x 2 directions.
  - Phase A: P = emb @ w_ih_dir.T + bias -> bf16 DRAM table [VPAD, 1024],
    gate-chunk column order [f f g g o o i i] with the g block pre-scaled
    by 2 (tanh(g) = 2*sigmoid(2g) - 1). Bias is folded in via a K=1
    ones-matmul; PSUM->SBUF evacuation runs on GpSimd to keep DVE free.
  - Phase B: the 64 rows split into two independent 32-row chains that are
    software-pipelined; everything lives in a TRANSPOSED layout
    (partition = hidden/gate unit mod 128, columns = (chunk, batch)), so
    the recurrence needs zero PE transposes:
      gates^T [128, 256] PSUM = xp^T (8 seed matmuls with lhsT=xp) +
        W_hh^T h (16 matmuls with lhsT=whh chunk, rhs=h^T slice)
      s = sigmoid(gates)  (one [128,256] Act op; g block pre-scaled)
      fc = f*c; w = (sg-0.5)*i; c' = 2w+fc  (scalar_tensor_tensor fusions)
      tc = tanh(c'); h^T = o*tc -> staged bf16, streamed to DRAM
  - Tokens for both steps of a "superstep" are gathered in ONE indirect
    DMA (128 indices) to halve the SWDGE fixed cost.
  - Host gathers rows at target_idx and runs the tiny output projection.
"""

import sys

for _p in ("/opt/trn_rl_repo",):
    if _p not in sys.path:
        sys.path.insert(0, _p)

import numpy as np
import ml_dtypes

import concourse.bass as bass
import concourse.bacc as bacc
import concourse.mybir as mybir
import concourse.tile as tile
from concourse.masks import make_identity

F32 = mybir.dt.float32
BF16 = mybir.dt.bfloat16
I32 = mybir.dt.int32
AF = mybir.ActivationFunctionType
ALU = mybir.AluOpType

H = 256
E = 768
V = 7987
B = 256
T = 512
G4 = 4 * H  # 1024
BS = 64  # batch rows per core
CH = 32  # rows per chain (2 chains per core)
VC = 63  # ceil(V/128)
VPAD = VC * 128  # 8064
NCORES = 8

# Gate-chunk order in the transposed layout: [f f | g g | i i | o o]
# (chunks of 128 gate units).  PyTorch row order is [i, f, g, o].  o goes
# last so the critical-path sigmoid only spans f/g/i.
_PERM = np.concatenate(
    [np.arange(256, 512), np.arange(512, 768), np.arange(0, 256), np.arange(768, 1024)]
)
_GSCL = np.ones(G4, dtype=np.float32)
_GSCL[256:512] = 2.0  # the g block (post-perm cols 256:512) pre-scaled by 2


def build_program(t_steps: int = T, chunk_sched=None, debug: bool = False):
    """chunk_sched[sp] = cumulative #vocab-chunks that must be emitted before
    superstep sp.  None = build the whole table up front."""
    nsup = t_steps // 2
    if chunk_sched is None:
        chunk_sched = [VC] * nsup

    nc = bacc.Bacc("TRN2", target_bir_lowering=False)
    if debug:
        dbg_xp = nc.declare_dram_parameter("dbg_xp", [128, G4], BF16, isOutput=True)
        dbg_s = nc.declare_dram_parameter("dbg_s", [4, 128, 256], F32, isOutput=True)
        dbg_c = nc.declare_dram_parameter("dbg_c", [4, 128, 64], F32, isOutput=True)

    tokg_p = nc.declare_dram_parameter("tokg", [128, nsup], I32, isOutput=False)
    embt_p = nc.declare_dram_parameter("embt", [VC, 128, 6 * 128], BF16, isOutput=False)
    wih_p = nc.declare_dram_parameter("wiht", [128, 6 * G4], BF16, isOutput=False)
    bias_p = nc.declare_dram_parameter("biasrow", [1, G4], BF16, isOutput=False)
    whh_p = nc.declare_dram_parameter("whht", [128, 2 * G4], BF16, isOutput=False)
    hout_p = nc.declare_dram_parameter("hout", [nsup, 128, 256], BF16, isOutput=True)
    p_tab = nc.dram_tensor("ptab", [VPAD, G4], BF16)

    with tile.TileContext(nc) as tc:
        with (
            tc.tile_pool(name="const", bufs=1) as cpool,
            tc.tile_pool(name="emba", bufs=3) as embp,
            tc.tile_pool(name="pouta", bufs=3) as poutp,
            tc.tile_pool(name="psa", bufs=2, space="PSUM") as psa,
            tc.tile_pool(name="xp", bufs=4) as xpp,
            tc.tile_pool(name="gps", bufs=2, space="PSUM") as gpsp,
            tc.tile_pool(name="sp", bufs=4) as spp,
            tc.tile_pool(name="dv", bufs=4) as dvp,
            tc.tile_pool(name="cs", bufs=4) as csp,
            tc.tile_pool(name="stg", bufs=3) as stgp,
        ):
            # ---- resident constants ----
            wih_sb = cpool.tile([128, 6 * G4], BF16)
            nc.sync.dma_start(wih_sb[:], wih_p[:])
            bias_sb = cpool.tile([1, G4], BF16)
            nc.sync.dma_start(bias_sb[:], bias_p[:])
            whh_sb = cpool.tile([128, 2 * G4], BF16)
            nc.sync.dma_start(whh_sb[:], whh_p[:])
            tokg_sb = cpool.tile([128, nsup], I32)
            nc.sync.dma_start(tokg_sb[:], tokg_p[:])
            # 64x64 identity replicated at partition bases 0 and 64: seed
            # matmuls run with K=64 (lhsT = a 64-row half of xp2, legal base
            # partitions 0/64) and the rhs column block selects which 32 rows
            # (chain) get extracted.
            identf = cpool.tile([128, 64], F32)
            for q in range(2):
                make_identity(nc, identf[q * 64 : (q + 1) * 64, :])
            identb = cpool.tile([128, 64], BF16)
            nc.vector.tensor_copy(identb[:], identf[:])
            ones1 = cpool.tile([1, 128], BF16)
            nc.gpsimd.memset(ones1[:], 1.0)
            zlhs = cpool.tile([1, 128], BF16)
            nc.gpsimd.memset(zlhs[:], 0.0)
            zrhs = cpool.tile([1, 256], BF16)
            nc.gpsimd.memset(zrhs[:], 0.0)

            # initial state: h = 0 (one staging-like tile), c = 0 per chain
            hzero = cpool.tile([128, 256], BF16)
            nc.gpsimd.memset(hzero[:], 0.0)
            czero = [cpool.tile([128, 64], F32, name=f"czero{i}") for i in range(2)]
            for z in czero:
                nc.gpsimd.memset(z[:], 0.0)

            # ---- table chunk emission (phase A work unit) ----
            def emit_chunk(vc):
                lhs = embp.tile([128, 6 * 128], BF16, tag="emb")
                nc.sync.dma_start(lhs[:], embt_p[vc])
                for ns in range(2):
                    ps = psa.tile([128, 512], F32, tag="pps")
                    # bias via K=1 ones-matmul (also sets has_written)
                    nc.tensor.matmul(
                        ps[:],
                        ones1[:],
                        bias_sb[:, ns * 512 : (ns + 1) * 512],
                        start=True,
                        stop=False,
                    )
                    for kc in range(6):
                        nc.tensor.matmul(
                            ps[:],
                            lhs[:, kc * 128 : (kc + 1) * 128],
                            wih_sb[:, kc * G4 + ns * 512 : kc * G4 + ns * 512 + 512],
                            start=False,
                            stop=(kc == 5),
                        )
                    po = poutp.tile([128, 512], BF16, tag="pout")
                    nc.vector.tensor_copy(po[:], ps[:])
                    nc.sync.dma_start(
                        p_tab[vc * 128 : (vc + 1) * 128, ns * 512 : (ns + 1) * 512],
                        po[:],
                    )

            # ---- phase B: software-pipelined, chain B offset half a step ----
            # per-chain state: (tile, col) for h^T, tile for c
            ht_prev = [(hzero, 0), (hzero, 64)]
            c_prev = [czero[0], czero[1]]
            chunks_done = 0
            PREFETCH = 2
            xp_tiles = {}
            stag_tiles = {}
            s_tiles = {}

            def emit_gather(spi):
                if spi >= nsup:
                    return
                xp2 = xpp.tile([128, G4], BF16, tag="xp")
                nc.gpsimd.indirect_dma_start(
                    out=xp2[:],
                    out_offset=None,
                    in_=p_tab[:, :],
                    in_offset=bass.IndirectOffsetOnAxis(
                        ap=tokg_sb[:, spi : spi + 1], axis=0
                    ),
                )
                xp_tiles[spi] = xp2

            def head(ch, t):
                """PSUM gates + sigmoid for chain ch, step t."""
                sp, ti = t // 2, t % 2
                xp2 = xp_tiles[sp]
                hbase = ti * 64
                g = gpsp.tile([128, 256], F32, tag=f"g{ch}", name=f"g{ch}_{t}")
                # exactly ONE start=True per PSUM bank (marks the whole 2KB
                # zero-region pending); later matmuls first-touch-replace,
                # then accumulate
                for c in range(8):
                    nc.tensor.matmul(
                        g[:, c * 32 : (c + 1) * 32],
                        xp2[hbase : hbase + 64, c * 128 : (c + 1) * 128],
                        identb[hbase : hbase + 64, ch * 32 : ch * 32 + 32],
                        start=(c == 0),
                        stop=False,
                        skip_group_check=True,
                    )
                htile, hcol = ht_prev[ch]
                # rec order: (kc0 then kc1) within f/g/i first, o-gate last —
                # kc0 only needs the first half of h, and the main sigmoid
                # doesn't wait on the o-gate regions at all
                for cs in ((0, 6), (6, 8)):
                    for kc in range(2):
                        for c in range(*cs):
                            nc.tensor.matmul(
                                g[:, c * 32 : (c + 1) * 32],
                                whh_sb[:, kc * G4 + c * 128 : kc * G4 + c * 128 + 128],
                                htile[:, hcol + kc * 32 : hcol + kc * 32 + 32],
                                start=False,
                                stop=(kc == 1 and c == cs[1] - 1),
                                skip_group_check=True,
                            )
                s = spp.tile([128, 256], F32, tag=f"s{ch}", name=f"s{ch}_{t}")
                nc.scalar.activation(s[:, 0:192], g[:, 0:192], AF.Sigmoid)
                nc.scalar.activation(s[:, 192:256], g[:, 192:256], AF.Sigmoid)
                s_tiles[(ch, t)] = s
                if debug and sp == 0:
                    nc.sync.dma_start(dbg_s[ti * 2 + ch], s[:])

            def tail(ch, t):
                """c/h update for chain ch, step t; h lands in the stage tile."""
                sp, ti = t // 2, t % 2
                s = s_tiles.pop((ch, t))
                fc = dvp.tile([128, 64], F32, tag=f"fc{ch}", name=f"fc{ch}_{t}")
                nc.vector.tensor_tensor(fc[:], s[:, 0:64], c_prev[ch][:], ALU.mult)
                w = dvp.tile([128, 64], F32, tag=f"w{ch}", name=f"w{ch}_{t}")
                nc.vector.scalar_tensor_tensor(
                    w[:], s[:, 64:128], 0.5, s[:, 128:192], ALU.subtract, ALU.mult
                )
                cn = csp.tile([128, 64], F32, tag=f"c{ch}", name=f"c{ch}_{t}")
                nc.vector.scalar_tensor_tensor(
                    cn[:], w[:], 2.0, fc[:], ALU.mult, ALU.add
                )
                if debug and sp == 0:
                    nc.sync.dma_start(dbg_c[ti * 2 + ch], cn[:])
                tcn = dvp.tile([128, 64], F32, tag=f"tc{ch}", name=f"tc{ch}_{t}")
                nc.scalar.activation(tcn[:], cn[:], AF.Tanh)
                if sp not in stag_tiles:
                    stag_tiles[sp] = stgp.tile(
                        [128, 256], BF16, tag="stg", name=f"stg_{sp}"
                    )
                stag = stag_tiles[sp]
                slot = ti * 2 + ch
                # h split by hidden chunk so the next step's kc0 rec matmuls
                # can start before the second half lands
                for kc in range(2):
                    nc.vector.tensor_tensor(
                        stag[:, slot * 64 + kc * 32 : slot * 64 + kc * 32 + 32],
                        s[:, 192 + kc * 32 : 224 + kc * 32],
                        tcn[:, kc * 32 : kc * 32 + 32],
                        ALU.mult,
                    )
                ht_prev[ch] = (stag, slot * 64)
                c_prev[ch] = cn

            A, Bc = 0, 1
            next_gather = 0

            def advance_gathers(upto):
                nonlocal chunks_done, next_gather
                while next_gather <= min(upto, nsup - 1):
                    need = chunk_sched[next_gather]
                    while chunks_done < need:
                        emit_chunk(chunks_done)
                        chunks_done += 1
                    emit_gather(next_gather)
                    next_gather += 1

            for t in range(t_steps + 1):
                if t < t_steps and t % 2 == 0:
                    advance_gathers(t // 2 + PREFETCH)
                if t == 0:
                    # table rows for the prefetched gathers are emitted
                    # first; then the pipeline prologue
                    head(A, 0)
                if t >= 1:
                    tail(Bc, t - 1)
                    if (t - 1) % 2 == 1:
                        spd = (t - 1) // 2
                        nc.sync.dma_start(hout_p[spd], stag_tiles.pop(spd)[:])
                        xp_tiles.pop(spd, None)
                if t < t_steps:
                    tail(A, t)
                    head(Bc, t)
                    if t + 1 < t_steps:
                        head(A, t + 1)

    nc.compile()
    return nc


def _plan_schedule(inputs, t_steps: int = T, ahead: int = 4):
    """First-use vocab ranking per shard + a shared chunk schedule.

    Returns (chunk_sched, ranks) where ranks[shard][v] = table row of vocab v
    for that shard's cores, and chunk_sched[sp] = cumulative chunks that must
    be emitted before superstep sp (covering every core's tokens through
    steps 2*(sp+ahead)+1).
    """
    nsup = t_steps // 2
    tokens = np.asarray(inputs["input_tensor"]).astype(np.int64)  # [B, T]
    nshard = B // BS
    ranks = []
    needs = np.zeros((nshard, t_steps), dtype=np.int64)  # distinct tokens thru t
    for s in range(nshard):
        tok = tokens[s * BS : (s + 1) * BS, :t_steps]  # [64, t]
        flat = tok.T.reshape(-1)  # step-major
        first_pos = np.full(VPAD, np.iinfo(np.int64).max, dtype=np.int64)
        # first occurrence position of each vocab id
        uniq, idx = np.unique(flat, return_index=True)
        first_pos[uniq] = idx
        order = np.argsort(first_pos, kind="stable")  # used vocab first
        rank = np.empty(VPAD, dtype=np.int64)
        rank[order] = np.arange(VPAD)
        ranks.append(rank)
        # need(t) = #vocab with first_pos < 64*(t+1)
        fp_used = np.sort(first_pos[uniq])
        needs[s] = np.searchsorted(fp_used, BS * (np.arange(t_steps) + 1))
    need_max = needs.max(axis=0)  # [t]
    chunk_sched = []
    for sp in range(nsup):
        t_cov = min(t_steps - 1, 2 * (sp + ahead) + 1)
        chunk_sched.append(min(VC, int(-(-need_max[t_cov] // 128)) + 1))
    return chunk_sched, ranks


def _prep_core_inputs(inputs, t_steps: int = T, ranks=None):
    """Build per-core input maps. Core c: dir = c % 2, shard = c // 2."""
    nsup = t_steps // 2
    tokens = np.asarray(inputs["input_tensor"]).astype(np.int32)  # [B, T]
    emb = np.asarray(inputs["emb"], dtype=np.float32)  # [V, E]

    embp = np.zeros((VPAD, E), dtype=np.float32)
    embp[:V] = emb

    def make_embt(ep):
        # embt[vc, p, kc*128+m] = ep[vc*128+m, kc*128+p]
        return np.ascontiguousarray(
            ep.reshape(VC, 128, 6, 128).transpose(0, 3, 2, 1).reshape(VC, 128, 6 * 128)
        ).astype(ml_dtypes.bfloat16)

    if ranks is None:
        embt_by_shard = [make_embt(embp)] * (B // BS)
    else:
        embt_by_shard = []
        for s in range(B // BS):
            order = np.argsort(ranks[s], kind="stable")  # order[r] = vocab at row r
            embt_by_shard.append(make_embt(embp[order]))

    per_dir = {}
    for d, sfx in ((0, "f"), (1, "b")):
        w_ih = np.asarray(inputs[f"w_ih_{sfx}"], dtype=np.float32)[_PERM]
        w_hh = np.asarray(inputs[f"w_hh_{sfx}"], dtype=np.float32)[_PERM]
        bias = (
            np.asarray(inputs[f"b_ih_{sfx}"], dtype=np.float32)
            + np.asarray(inputs[f"b_hh_{sfx}"], dtype=np.float32)
        )[_PERM]
        w_ih = w_ih * _GSCL[:, None]
        w_hh = w_hh * _GSCL[:, None]
        bias = bias * _GSCL
        # wiht[p, kc*G4+j] = w_ih[j, kc*128+p]
        wiht = np.ascontiguousarray(
            w_ih.T.reshape(6, 128, G4).transpose(1, 0, 2).reshape(128, 6 * G4)
        ).astype(ml_dtypes.bfloat16)
        # whht[k, kc*G4+j] = w_hh[j, kc*128+k]
        whht = np.ascontiguousarray(
            w_hh.T.reshape(2, 128, G4).transpose(1, 0, 2).reshape(128, 2 * G4)
        ).astype(ml_dtypes.bfloat16)
        biasrow = np.ascontiguousarray(bias[None, :]).astype(ml_dtypes.bfloat16)
        per_dir[d] = (wiht, whht, biasrow)

    in_maps = []
    for c in range(NCORES):
        d, s = c % 2, c // 2
        wiht, whht, biasrow = per_dir[d]
        tok = tokens[s * BS : (s + 1) * BS, :t_steps].astype(np.int64)  # [64, t]
        if ranks is not None:
            tok = ranks[s][tok]
        # tokg[p, sp]: idx p of superstep sp = [rows tok(2sp); rows tok(2sp+1)]
        tokg = np.ascontiguousarray(tok.T.reshape(nsup, 128).T).astype(np.int32)
        in_maps.append(
            {
                "tokg": tokg,
                "embt": embt_by_shard[s],
                "wiht": wiht,
                "biasrow": biasrow,
                "whht": whht,
            }
        )
    return in_maps


_PROGRAM_CACHE = {}
INTERLEAVE = True  # build the vocab table concurrently with the recurrence


def run_on_hw(inputs, **spmd_kwargs):
    """Run the SPMD program; returns BassKernelResults."""
    from concourse.bass_utils import run_bass_kernel_spmd

    t_steps = T
    assert int(np.asarray(inputs["max_length"])) == T

    if INTERLEAVE:
        chunk_sched, ranks = _plan_schedule(inputs, t_steps)
    else:
        chunk_sched, ranks = None, None

    key = (t_steps, tuple(chunk_sched) if chunk_sched else None)
    if key not in _PROGRAM_CACHE:
        _PROGRAM_CACHE[key] = build_program(t_steps, chunk_sched)
    nc = _PROGRAM_CACHE[key]

    in_maps = _prep_core_inputs(inputs, t_steps, ranks)
    res = run_bass_kernel_spmd(nc, in_maps, list(range(NCORES)), **spmd_kwargs)
    return res


def postprocess(inputs, res) -> np.ndarray:
    tgt = np.asarray(inputs["target_idx"]).astype(np.int64)  # [B]
    w_out = np.asarray(inputs["w_out"], dtype=np.float32)  # [1, 2H]
    b_out = np.asarray(inputs["b_out"], dtype=np.float32)  # [1]

    gathered = np.empty((B, 2 * H), dtype=np.float32)
    for c in range(NCORES):
        d, s = c % 2, c // 2
        # hout[sp, p, (ti*2+ch)*64 + kc*32 + b] = h[ch*32+b, kc*128+p] at t=2sp+ti
        hout = np.asarray(res.results[c]["hout"]).astype(np.float32)  # [nsup,128,256]
        hr = hout.reshape(T // 2, 128, 2, 2, 2, 32)  # [sp, p, ti, ch, kc, b]
        for r in range(BS):
            t = tgt[s * BS + r]
            spi, ti = int(t) // 2, int(t) % 2
            ch, b = r // 32, r % 32
            # h vector: hidden = kc*128+p
            hv = hr[spi, :, ti, ch, :, b]  # [p, kc]
            gathered[s * BS + r, d * H : (d + 1) * H] = hv.T.reshape(H)

    logits = gathered @ w_out.T + b_out  # [B, 1]
    out = 1.0 / (1.0 + np.exp(-logits))
    return out[:, None, :].astype(np.float32)  # [B, 1, 1]


def kernel(**inputs) -> np.ndarray:
    res = run_on_hw(inputs)
    return postprocess(inputs, res)
